# revision 28
# baseline (speedup 1.0000x reference)
"""Distributed KNN retrieval kernel for Trainium2 (8 NeuronCores).

Problem: per-region cosine top-k over a bank of N=8000 vectors,
  query: [B=16, R=29, H=768], bank: [R=29, N=8000, H=768],
  exclude_idx: [B, R], top_k=5.

Strategy (standard distributed ANN):
  - Shard the bank along N across 8 cores (1024 columns/core, zero-padded
    from 8000). The per-core shard is pre-transposed and cast to
    fp8-e4m3 (x16 scale) on the host (one jitted jax-CPU call), so the
    device streams it with plain contiguous DMAs at full HBM rate.
  - Each core: fp8 matmul of the normalized queries against its bank
    shard (unnormalized approximate scores), then a DVE top-16 per
    (b, r) row (max/max_index, match_replace -inf, max/max_index).
  - Host: gather the 8*16 candidates per row, drop the excluded index,
    exactly rescore the <=128 candidates in float64 (true cosine), take
    the global top-k.

Device layout details:
  - Regions are processed in PAIRS with a stacked contraction of 2H=1536
    (12 k-tiles): lhsT is [128, 32] with the off-region half zeroed, so
    one PSUM tile holds [32, 512] rows for two regions and all SBUF
    partition offsets stay 32-aligned (HW requires base partition in
    {0, 32, 64, 96}).
  - R=29 is padded with one zero region to 15 pairs; its rows land above
    row 464 and are ignored by the host.

Correctness of candidate containment: ranking by unnormalized fp8
scores instead of exact cosine perturbs scores by ~1.3e-3 (fp8 quant
noise) + ~2.3e-3 (bank-norm spread) while the mean adjacent rank gap
near the top is ~1e-2; losing a true top-5 candidate from a per-core
top-16 needs a >=11-rank shift, whose probability is ~Poisson tail
(mu~0.3, k>=11) ~ 1e-13 per row. The exact host rescore then restores
full f32/f64 precision for both values and ordering.
"""

import numpy as np
import ml_dtypes

R, B, H = 29, 16, 768
KT = H // 128     # 6 contraction tiles per region
KT2 = 2 * KT      # 12 contraction tiles per region-pair
PAIRS = R // 2    # 14 full pairs; region 28 is processed alone
RPAD = R          # 29 regions, no padding
S = 1024          # bank columns per core (8 * 1024 = 8192 >= 8000, zero padded)
N = 8000
NCORES = 8
KEEP = 16         # candidates kept per core per row
ROWS = B * R                    # 464 real (b, r) rows
ROWT = (RPAD * B + 127) // 128  # 4 row-tiles of 128 partitions
FP8_SCALE = 16.0
EPS = 1e-12

_PROGRAM_CACHE = {}
LAST_RESULTS = None  # BassKernelResults of the most recent device run


def _install_ntff_hook_shim():
    """Provide antenv.axon_hooks if the image lacks it, so
    run_bass_kernel_spmd(trace=True) can capture NTFF profiles.
    Best-effort: silently does nothing if unavailable."""
    import sys

    try:
        from antenv.axon_hooks import get_axon_ntff_profile_hook  # noqa: F401

        return
    except ImportError:
        pass
    import contextlib
    import ctypes
    import glob
    import types

    so_candidates = glob.glob("/opt/axon/libaxon_pjrt.so") or glob.glob(
        "/opt/**/libaxon_pjrt.so", recursive=True
    )
    hook = None
    if so_candidates:
        try:
            lib = ctypes.CDLL(so_candidates[0])
            lib.axon_start_nrt_profile.argtypes = [
                ctypes.POINTER(ctypes.c_int64),
                ctypes.c_size_t,
            ]
            lib.axon_start_nrt_profile.restype = ctypes.c_int64
            lib.axon_stop_nrt_profile.argtypes = [ctypes.c_char_p]
            lib.axon_stop_nrt_profile.restype = ctypes.c_int64

            @contextlib.contextmanager
            def _hook(output_dir, device_ids):
                import jax

                jax.devices()
                if device_ids:
                    ids = (ctypes.c_int64 * len(device_ids))(*device_ids)
                    rc = lib.axon_start_nrt_profile(ids, len(device_ids))
                else:
                    rc = lib.axon_start_nrt_profile(None, 0)
                if rc != 0:
                    raise RuntimeError(f"axon_start_nrt_profile rc={rc}")
                try:
                    yield
                finally:
                    n = lib.axon_stop_nrt_profile(str(output_dir).encode())
                    print(f"ntff profile: {n} file(s) -> {output_dir}")

            hook = _hook
        except (OSError, AttributeError):
            hook = None

    mod = types.ModuleType("antenv.axon_hooks")
    mod._hook = hook
    mod.get_axon_ntff_profile_hook = lambda: mod._hook

    def _set(h):
        mod._hook = h

    mod.set_axon_ntff_profile_hook = _set
    sys.modules["antenv.axon_hooks"] = mod
    try:
        import antenv

        antenv.axon_hooks = mod
    except ImportError:
        pass


_install_ntff_hook_shim()


def _build_program():
    from contextlib import ExitStack
    from concourse import bacc, tile, mybir

    fp8 = mybir.dt.float8e4
    f32 = mybir.dt.float32
    u32 = mybir.dt.uint32

    nc = bacc.Bacc(
        "TRN2",
        target_bir_lowering=False,
        debug=False,
        num_devices=NCORES,
    )

    # qT2[p, (m*KT2 + kt2)*32 + c]: for kt2 < KT columns 0..15 hold
    # qhat[b, 2m, (kt2%KT)*128 + p] (cols 16..31 zero); for kt2 >= KT
    # columns 16..31 hold qhat[b, 2m+1, ...] (cols 0..15 zero).
    q_cols = PAIRS * KT2 * 32 + KT * B  # paired blocks + single-region block
    qT_ext = nc.declare_dram_parameter("qT", [128, q_cols], fp8, isOutput=False)
    # bank[r, p, kt*S + n] = shard[r, n, kt*128 + p] * FP8_SCALE
    # (pre-transposed on host; each region is one contiguous [128, KT*S]
    #  plain DMA at full HBM bandwidth)
    bank_ext = nc.declare_dram_parameter(
        "bank", [RPAD, 128, KT * S], fp8, isOutput=False
    )
    vals_ext = nc.declare_dram_parameter(
        "out_vals", [ROWT, 128, KEEP], f32, isOutput=True
    )
    idx_ext = nc.declare_dram_parameter(
        "out_idx", [ROWT, 128, KEEP], u32, isOutput=True
    )

    with ExitStack() as ctx:
        tc = ctx.enter_context(tile.TileContext(nc))
        qpool = ctx.enter_context(tc.tile_pool(name="q", bufs=1))
        bpool = ctx.enter_context(tc.tile_pool(name="bankT", bufs=6))
        spool = ctx.enter_context(tc.tile_pool(name="sims", bufs=1))
        opool = ctx.enter_context(tc.tile_pool(name="outs", bufs=2))
        ppool = ctx.enter_context(tc.tile_pool(name="psum", bufs=6, space="PSUM"))

        qT = qpool.tile([128, q_cols], fp8)
        nc.scalar.dma_start(out=qT[:], in_=qT_ext[:])

        sims = [
            spool.tile([128, S], f32, name=f"sims{t}", tag=f"sims{t}")
            for t in range(ROWT)
        ]
        # partitions above the last written row (R*B = 464) are never
        # copied into; zero them so the tail top-16 reads defined memory
        # (zero from 64: 32-aligned; rows 64..79 are overwritten by copies)
        nc.gpsimd.memset(sims[ROWT - 1][64:, :], 0.0)

        for m in range(PAIRS):
            # one plain DMA per pair (1.5 MB), alternating HWDGE rings
            bt = bpool.tile([128, 2, KT, S], fp8, tag="bankT")
            dma_eng = nc.sync if m % 2 == 0 else nc.scalar
            dma_eng.dma_start(
                out=bt[:],
                in_=bank_ext[2 * m : 2 * m + 2].rearrange("a p f -> p a f"),
            )
            r0 = 2 * m
            t, j = divmod(r0, 8)
            for half in range(2):
                ps = ppool.tile([32, 512], f32, tag="ps")
                # fp8 DoubleRow: each matmul contracts 256 h (2 adjacent
                # 128-chunks stacked on the AP's middle dim)
                for ktd in range(KT2 // 2):
                    kt2 = 2 * ktd
                    q_col = (m * KT2 + kt2) * 32
                    rp, kt = divmod(kt2, KT)
                    lhsT = qT[:, q_col : q_col + 64].rearrange(
                        "p (a b) -> p a b", a=2
                    )
                    nc.tensor.matmul(
                        ps[:],
                        lhsT,
                        bt[:, rp, kt : kt + 2, half * 512 : (half + 1) * 512],
                        start=(ktd == 0),
                        stop=(ktd == KT2 // 2 - 1),
                        perf_mode=mybir.MatmulPerfMode.DoubleRow,
                    )
                nc.vector.tensor_copy(
                    sims[t][j * B : j * B + 32, half * 512 : (half + 1) * 512],
                    ps[:],
                )

        # last region (r = R-1 = 28) processed alone: M=16 matmuls into
        # sims[3] rows 64..79
        r_s = 2 * PAIRS
        bt_s = bpool.tile([128, KT, S], fp8, tag="bankS", bufs=1)
        nc.sync.dma_start(out=bt_s[:], in_=bank_ext[r_s])
        q_base = PAIRS * KT2 * 32
        t_s, j_s = divmod(r_s, 8)
        for half in range(2):
            ps = ppool.tile([16, 512], f32, name="ps_s", tag="ps")
            for ktd in range(KT // 2):
                q_col = q_base + (2 * ktd) * B
                lhsT = qT[:, q_col : q_col + 2 * B].rearrange(
                    "p (a b) -> p a b", a=2
                )
                nc.tensor.matmul(
                    ps[:],
                    lhsT,
                    bt_s[:, 2 * ktd : 2 * ktd + 2, half * 512 : (half + 1) * 512],
                    start=(ktd == 0),
                    stop=(ktd == KT // 2 - 1),
                    perf_mode=mybir.MatmulPerfMode.DoubleRow,
                )
            nc.vector.tensor_copy(
                sims[t_s][j_s * B : j_s * B + 16, half * 512 : (half + 1) * 512],
                ps[:],
            )

        for t in range(ROWT):
            mx = opool.tile([128, KEEP], f32, tag="mx")
            ix = opool.tile([128, KEEP], u32, tag="ix")
            rep = opool.tile([128, S], f32, tag="rep")
            nc.vector.max(mx[:, 0:8], sims[t][:])
            nc.vector.max_index(ix[:, 0:8], mx[:, 0:8], sims[t][:])
            nc.vector.match_replace(rep[:], mx[:, 0:8], sims[t][:], -1.0e30)
            nc.vector.max(mx[:, 8:16], rep[:])
            nc.vector.max_index(ix[:, 8:16], mx[:, 8:16], rep[:])
            nc.sync.dma_start(out=vals_ext[t], in_=mx[:])
            nc.sync.dma_start(out=idx_ext[t], in_=ix[:])

    nc.compile()
    return nc


def _get_program():
    if "nc" not in _PROGRAM_CACHE:
        _PROGRAM_CACHE["nc"] = _build_program()
    return _PROGRAM_CACHE["nc"]


_PREP_CACHE = {}


def _bank_prep_fn():
    """jitted jax-CPU function: pad + scale + fp8-cast + transpose the
    full bank into per-core pre-transposed shards."""
    if "fn" in _PREP_CACHE:
        return _PREP_CACHE["fn"]
    import jax
    import jax.numpy as jnp

    def prep(bank):
        x = jnp.pad(bank, ((0, 0), (0, NCORES * S - N), (0, 0)))
        x = (x * FP8_SCALE).astype(jnp.float8_e4m3)
        # [R, NCORES, S, KT, 128] -> [NCORES, R, 128, KT, S]
        x = x.reshape(R, NCORES, S, KT, 128)
        x = x.transpose(1, 0, 4, 3, 2)
        return x.reshape(NCORES, R, 128, KT * S)

    _PREP_CACHE["fn"] = jax.jit(prep, backend="cpu")
    return _PREP_CACHE["fn"]


def make_in_maps(query, bank):
    """Host-side sharding: normalized fp8 query layout + fp8 shards."""
    q64 = np.asarray(query, dtype=np.float64)
    qn = q64 / np.maximum(np.linalg.norm(q64, axis=2, keepdims=True), EPS)

    # per-region transposed queries: qTr[p, r, kt, b] = qhat[b, r, kt*128+p]
    qTr = qn.astype(np.float32).reshape(B, R, KT, 128).transpose(3, 1, 2, 0)
    qT2 = np.zeros((128, PAIRS, KT2, 32), dtype=np.float32)
    for m in range(PAIRS):
        qT2[:, m, :KT, :B] = qTr[:, 2 * m]
        qT2[:, m, KT:, B:] = qTr[:, 2 * m + 1]
    q_single = qTr[:, 2 * PAIRS].reshape(128, KT * B)  # last region, alone
    qT_f = np.concatenate(
        [qT2.reshape(128, PAIRS * KT2 * 32), q_single], axis=1
    )
    qT_b = (qT_f * FP8_SCALE).astype(ml_dtypes.float8_e4m3)

    bank = np.asarray(bank, dtype=np.float32)
    shards = np.asarray(_bank_prep_fn()(bank))  # [NCORES, RPAD, 128, KT*S]
    in_maps = [{"qT": qT_b, "bank": shards[c]} for c in range(NCORES)]
    return in_maps, qn


def kernel(query, bank, exclude_idx, top_k):
    global LAST_RESULTS
    from concourse.bass_utils import run_bass_kernel_spmd

    query = np.asarray(query, dtype=np.float32)
    bank = np.asarray(bank, dtype=np.float32)
    exclude_idx = np.asarray(exclude_idx).astype(np.int64)
    k = int(np.asarray(top_k))
    assert k <= KEEP - 1, f"device keeps top-{KEEP} per core; top_k={k} too large"

    in_maps, qn = make_in_maps(query, bank)

    nc = _get_program()
    LAST_RESULTS = run_bass_kernel_spmd(nc, in_maps, core_ids=list(range(NCORES)))
    res = LAST_RESULTS.results

    # ---- host-side merge: candidates per (r, b) row ----
    # device row packing: flat row index == r*B + b
    ncand = NCORES * KEEP
    cand = np.empty((R, B, ncand), dtype=np.int64)
    for c in range(NCORES):
        idx = (
            res[c]["out_idx"]
            .reshape(ROWT * 128, KEEP)[:ROWS]
            .reshape(R, B, KEEP)
            .astype(np.int64)
        )
        real = min((c + 1) * S, N) - c * S
        g = c * S + idx
        g[idx >= real] = -1  # zero padding or tie fallback (-1) slots
        cand[:, :, c * KEEP : (c + 1) * KEEP] = g

    # ---- exact rescoring in float64, region by region ----
    top_sims = np.empty((R, B, k), dtype=np.float32)
    top_idx = np.empty((R, B, k), dtype=np.int32)
    for r in range(R):
        cr = cand[r]  # [B, ncand]
        safe = np.clip(cr, 0, N - 1)
        v = bank[r][safe].astype(np.float64)  # [B, ncand, H]
        nv = np.maximum(np.linalg.norm(v, axis=2), EPS)
        scores = np.einsum("bch,bh->bc", v, qn[:, r]) / nv

        invalid = (cr < 0) | (cr == exclude_idx[:, r, None])
        # order candidates by index so ties resolve to the lowest index,
        # and drop duplicate indices (possible only via exact score ties)
        perm = np.argsort(cr, axis=1, kind="stable")
        cr_s = np.take_along_axis(cr, perm, axis=1)
        sc_s = np.take_along_axis(scores, perm, axis=1)
        inv_s = np.take_along_axis(invalid, perm, axis=1)
        inv_s[:, 1:] |= cr_s[:, 1:] == cr_s[:, :-1]
        sc_s[inv_s] = -np.inf

        order = np.argsort(-sc_s, axis=1, kind="stable")[:, :k]
        top_idx[r] = np.take_along_axis(cr_s, order, axis=1)
        top_sims[r] = np.take_along_axis(sc_s, order, axis=1)

    top_sims = np.ascontiguousarray(top_sims.transpose(1, 0, 2))  # [B, R, k]
    top_idx = np.ascontiguousarray(top_idx.transpose(1, 0, 2))
    best_sims = np.ascontiguousarray(top_sims[:, :, 0])
    return top_sims, top_idx.astype(np.int32), best_sims


# revision 29
# speedup vs baseline: 1.2287x; 1.2287x over previous
"""Distributed KNN retrieval kernel for Trainium2 (8 NeuronCores).

Problem: per-region cosine top-k over a bank of N=8000 vectors,
  query: [B=16, R=29, H=768], bank: [R=29, N=8000, H=768],
  exclude_idx: [B, R], top_k=5.

Strategy (standard distributed ANN):
  - Shard the bank along N across 8 cores (1024 columns/core, zero-padded
    from 8000). The per-core shard is pre-transposed and cast to
    fp8-e4m3 (x16 scale) on the host (one jitted jax-CPU call), so the
    device streams it with plain contiguous DMAs at full HBM rate.
  - Each core: fp8 matmul of the normalized queries against its bank
    shard (unnormalized approximate scores), then a DVE top-16 per
    (b, r) row (max/max_index, match_replace -inf, max/max_index).
  - Host: gather the 8*16 candidates per row, drop the excluded index,
    exactly rescore the <=128 candidates in float64 (true cosine), take
    the global top-k.

Device layout details:
  - Regions are processed in PAIRS with a stacked contraction of 2H=1536
    (12 k-tiles): lhsT is [128, 32] with the off-region half zeroed, so
    one PSUM tile holds [32, 512] rows for two regions and all SBUF
    partition offsets stay 32-aligned (HW requires base partition in
    {0, 32, 64, 96}).
  - R=29 is padded with one zero region to 15 pairs; its rows land above
    row 464 and are ignored by the host.

Correctness of candidate containment: ranking by unnormalized fp8
scores instead of exact cosine perturbs scores by ~1.3e-3 (fp8 quant
noise) + ~2.3e-3 (bank-norm spread) while the mean adjacent rank gap
near the top is ~1e-2; losing a true top-5 candidate from a per-core
top-16 needs a >=11-rank shift, whose probability is ~Poisson tail
(mu~0.3, k>=11) ~ 1e-13 per row. The exact host rescore then restores
full f32/f64 precision for both values and ordering.
"""

import numpy as np
import ml_dtypes

R, B, H = 29, 16, 768
KT = H // 128     # 6 contraction tiles per region
KT2 = 2 * KT      # 12 contraction tiles per region-pair
PAIRS = R // 2    # 14 full pairs; region 28 is processed alone
RPAD = R          # 29 regions, no padding
S = 1024          # bank columns per core (8 * 1024 = 8192 >= 8000, zero padded)
N = 8000
NCORES = 8
KEEP = 16         # candidates kept per core per row
ROWS = B * R                    # 464 real (b, r) rows
ROWT = (RPAD * B + 127) // 128  # 4 row-tiles of 128 partitions
FP8_SCALE = 16.0
EPS = 1e-12

_PROGRAM_CACHE = {}
LAST_RESULTS = None  # BassKernelResults of the most recent device run


def _install_ntff_hook_shim():
    """Provide antenv.axon_hooks if the image lacks it, so
    run_bass_kernel_spmd(trace=True) can capture NTFF profiles.
    Best-effort: silently does nothing if unavailable."""
    import sys

    try:
        from antenv.axon_hooks import get_axon_ntff_profile_hook  # noqa: F401

        return
    except ImportError:
        pass
    import contextlib
    import ctypes
    import glob
    import types

    so_candidates = glob.glob("/opt/axon/libaxon_pjrt.so") or glob.glob(
        "/opt/**/libaxon_pjrt.so", recursive=True
    )
    hook = None
    if so_candidates:
        try:
            lib = ctypes.CDLL(so_candidates[0])
            lib.axon_start_nrt_profile.argtypes = [
                ctypes.POINTER(ctypes.c_int64),
                ctypes.c_size_t,
            ]
            lib.axon_start_nrt_profile.restype = ctypes.c_int64
            lib.axon_stop_nrt_profile.argtypes = [ctypes.c_char_p]
            lib.axon_stop_nrt_profile.restype = ctypes.c_int64

            @contextlib.contextmanager
            def _hook(output_dir, device_ids):
                import jax

                jax.devices()
                if device_ids:
                    ids = (ctypes.c_int64 * len(device_ids))(*device_ids)
                    rc = lib.axon_start_nrt_profile(ids, len(device_ids))
                else:
                    rc = lib.axon_start_nrt_profile(None, 0)
                if rc != 0:
                    raise RuntimeError(f"axon_start_nrt_profile rc={rc}")
                try:
                    yield
                finally:
                    n = lib.axon_stop_nrt_profile(str(output_dir).encode())
                    print(f"ntff profile: {n} file(s) -> {output_dir}")

            hook = _hook
        except (OSError, AttributeError):
            hook = None

    mod = types.ModuleType("antenv.axon_hooks")
    mod._hook = hook
    mod.get_axon_ntff_profile_hook = lambda: mod._hook

    def _set(h):
        mod._hook = h

    mod.set_axon_ntff_profile_hook = _set
    sys.modules["antenv.axon_hooks"] = mod
    try:
        import antenv

        antenv.axon_hooks = mod
    except ImportError:
        pass


_install_ntff_hook_shim()


def _build_program():
    from contextlib import ExitStack
    from concourse import bacc, tile, mybir

    fp8 = mybir.dt.float8e4
    f32 = mybir.dt.float32
    u32 = mybir.dt.uint32

    nc = bacc.Bacc(
        "TRN2",
        target_bir_lowering=False,
        debug=False,
        num_devices=NCORES,
    )

    # qT2[p, (m*KT2 + kt2)*32 + c]: for kt2 < KT columns 0..15 hold
    # qhat[b, 2m, (kt2%KT)*128 + p] (cols 16..31 zero); for kt2 >= KT
    # columns 16..31 hold qhat[b, 2m+1, ...] (cols 0..15 zero).
    q_cols = PAIRS * KT2 * 32 + KT * B  # paired blocks + single-region block
    qT_ext = nc.declare_dram_parameter("qT", [128, q_cols], fp8, isOutput=False)
    # bank[r, p, kt*S + n] = shard[r, n, kt*128 + p] * FP8_SCALE
    # (pre-transposed on host; each region is one contiguous [128, KT*S]
    #  plain DMA at full HBM bandwidth)
    bank_ext = nc.declare_dram_parameter(
        "bank", [RPAD, 128, KT * S], fp8, isOutput=False
    )
    vals_ext = nc.declare_dram_parameter(
        "out_vals", [ROWT, 128, KEEP], f32, isOutput=True
    )
    idx_ext = nc.declare_dram_parameter(
        "out_idx", [ROWT, 128, KEEP], u32, isOutput=True
    )

    with ExitStack() as ctx:
        tc = ctx.enter_context(tile.TileContext(nc))
        qpool = ctx.enter_context(tc.tile_pool(name="q", bufs=1))
        bpool = ctx.enter_context(tc.tile_pool(name="bankT", bufs=5))
        spool = ctx.enter_context(tc.tile_pool(name="sims", bufs=1))
        opool = ctx.enter_context(tc.tile_pool(name="outs", bufs=2))
        ppool = ctx.enter_context(tc.tile_pool(name="psum", bufs=4, space="PSUM"))

        qT = qpool.tile([128, q_cols], fp8)
        nc.scalar.dma_start(out=qT[:], in_=qT_ext[:])

        sims = [
            spool.tile([128, S], f32, name=f"sims{t}", tag=f"sims{t}")
            for t in range(ROWT)
        ]
        # partitions above the last written row (R*B = 464) are never
        # copied into; zero them so the tail top-16 reads defined memory
        # (zero from 64: 32-aligned; rows 64..79 are overwritten by copies)
        nc.gpsimd.memset(sims[ROWT - 1][64:, :], 0.0)

        for m in range(PAIRS):
            # one plain DMA per pair (1.5 MB), alternating HWDGE rings
            bt = bpool.tile([128, 2, KT, S], fp8, tag="bankT")
            nc.sync.dma_start(
                out=bt[:],
                in_=bank_ext[2 * m : 2 * m + 2].rearrange("a p f -> p a f"),
            )
            r0 = 2 * m
            t, j = divmod(r0, 8)
            for half in range(2):
                ps = ppool.tile([32, 512], f32, tag="ps")
                # fp8 DoubleRow: each matmul contracts 256 h (2 adjacent
                # 128-chunks stacked on the AP's middle dim)
                for ktd in range(KT2 // 2):
                    kt2 = 2 * ktd
                    q_col = (m * KT2 + kt2) * 32
                    rp, kt = divmod(kt2, KT)
                    lhsT = qT[:, q_col : q_col + 64].rearrange(
                        "p (a b) -> p a b", a=2
                    )
                    nc.tensor.matmul(
                        ps[:],
                        lhsT,
                        bt[:, rp, kt : kt + 2, half * 512 : (half + 1) * 512],
                        start=(ktd == 0),
                        stop=(ktd == KT2 // 2 - 1),
                        perf_mode=mybir.MatmulPerfMode.DoubleRow,
                    )
                nc.scalar.activation(
                    sims[t][j * B : j * B + 32, half * 512 : (half + 1) * 512],
                    ps[:],
                    mybir.ActivationFunctionType.Copy,
                )

        # last region (r = R-1 = 28) processed alone: M=16 matmuls into
        # sims[3] rows 64..79
        r_s = 2 * PAIRS
        bt_s = bpool.tile([128, KT, S], fp8, tag="bankS", bufs=1)
        nc.sync.dma_start(out=bt_s[:], in_=bank_ext[r_s])
        q_base = PAIRS * KT2 * 32
        t_s, j_s = divmod(r_s, 8)
        for half in range(2):
            ps = ppool.tile([16, 512], f32, name="ps_s", tag="ps")
            for ktd in range(KT // 2):
                q_col = q_base + (2 * ktd) * B
                lhsT = qT[:, q_col : q_col + 2 * B].rearrange(
                    "p (a b) -> p a b", a=2
                )
                nc.tensor.matmul(
                    ps[:],
                    lhsT,
                    bt_s[:, 2 * ktd : 2 * ktd + 2, half * 512 : (half + 1) * 512],
                    start=(ktd == 0),
                    stop=(ktd == KT // 2 - 1),
                    perf_mode=mybir.MatmulPerfMode.DoubleRow,
                )
            nc.scalar.activation(
                sims[t_s][j_s * B : j_s * B + 16, half * 512 : (half + 1) * 512],
                ps[:],
                mybir.ActivationFunctionType.Copy,
            )

        for t in range(ROWT):
            mx = opool.tile([128, KEEP], f32, tag="mx")
            ix = opool.tile([128, KEEP], u32, tag="ix")
            rep = opool.tile([128, S], f32, tag="rep")
            nc.vector.max(mx[:, 0:8], sims[t][:])
            nc.vector.max_index(ix[:, 0:8], mx[:, 0:8], sims[t][:])
            nc.vector.match_replace(rep[:], mx[:, 0:8], sims[t][:], -1.0e30)
            nc.vector.max(mx[:, 8:16], rep[:])
            nc.vector.max_index(ix[:, 8:16], mx[:, 8:16], rep[:])
            nc.sync.dma_start(out=vals_ext[t], in_=mx[:])
            nc.sync.dma_start(out=idx_ext[t], in_=ix[:])

    nc.compile()
    return nc


def _get_program():
    if "nc" not in _PROGRAM_CACHE:
        _PROGRAM_CACHE["nc"] = _build_program()
    return _PROGRAM_CACHE["nc"]


_PREP_CACHE = {}


def _bank_prep_fn():
    """jitted jax-CPU function: pad + scale + fp8-cast + transpose the
    full bank into per-core pre-transposed shards."""
    if "fn" in _PREP_CACHE:
        return _PREP_CACHE["fn"]
    import jax
    import jax.numpy as jnp

    def prep(bank):
        x = jnp.pad(bank, ((0, 0), (0, NCORES * S - N), (0, 0)))
        x = (x * FP8_SCALE).astype(jnp.float8_e4m3)
        # [R, NCORES, S, KT, 128] -> [NCORES, R, 128, KT, S]
        x = x.reshape(R, NCORES, S, KT, 128)
        x = x.transpose(1, 0, 4, 3, 2)
        return x.reshape(NCORES, R, 128, KT * S)

    _PREP_CACHE["fn"] = jax.jit(prep, backend="cpu")
    return _PREP_CACHE["fn"]


def make_in_maps(query, bank):
    """Host-side sharding: normalized fp8 query layout + fp8 shards."""
    q64 = np.asarray(query, dtype=np.float64)
    qn = q64 / np.maximum(np.linalg.norm(q64, axis=2, keepdims=True), EPS)

    # per-region transposed queries: qTr[p, r, kt, b] = qhat[b, r, kt*128+p]
    qTr = qn.astype(np.float32).reshape(B, R, KT, 128).transpose(3, 1, 2, 0)
    qT2 = np.zeros((128, PAIRS, KT2, 32), dtype=np.float32)
    for m in range(PAIRS):
        qT2[:, m, :KT, :B] = qTr[:, 2 * m]
        qT2[:, m, KT:, B:] = qTr[:, 2 * m + 1]
    q_single = qTr[:, 2 * PAIRS].reshape(128, KT * B)  # last region, alone
    qT_f = np.concatenate(
        [qT2.reshape(128, PAIRS * KT2 * 32), q_single], axis=1
    )
    qT_b = (qT_f * FP8_SCALE).astype(ml_dtypes.float8_e4m3)

    bank = np.asarray(bank, dtype=np.float32)
    shards = np.asarray(_bank_prep_fn()(bank))  # [NCORES, RPAD, 128, KT*S]
    in_maps = [{"qT": qT_b, "bank": shards[c]} for c in range(NCORES)]
    return in_maps, qn


def kernel(query, bank, exclude_idx, top_k):
    global LAST_RESULTS
    from concourse.bass_utils import run_bass_kernel_spmd

    query = np.asarray(query, dtype=np.float32)
    bank = np.asarray(bank, dtype=np.float32)
    exclude_idx = np.asarray(exclude_idx).astype(np.int64)
    k = int(np.asarray(top_k))
    assert k <= KEEP - 1, f"device keeps top-{KEEP} per core; top_k={k} too large"

    in_maps, qn = make_in_maps(query, bank)

    nc = _get_program()
    LAST_RESULTS = run_bass_kernel_spmd(nc, in_maps, core_ids=list(range(NCORES)))
    res = LAST_RESULTS.results

    # ---- host-side merge: candidates per (r, b) row ----
    # device row packing: flat row index == r*B + b
    ncand = NCORES * KEEP
    cand = np.empty((R, B, ncand), dtype=np.int64)
    for c in range(NCORES):
        idx = (
            res[c]["out_idx"]
            .reshape(ROWT * 128, KEEP)[:ROWS]
            .reshape(R, B, KEEP)
            .astype(np.int64)
        )
        real = min((c + 1) * S, N) - c * S
        g = c * S + idx
        g[idx >= real] = -1  # zero padding or tie fallback (-1) slots
        cand[:, :, c * KEEP : (c + 1) * KEEP] = g

    # ---- exact rescoring in float64, region by region ----
    top_sims = np.empty((R, B, k), dtype=np.float32)
    top_idx = np.empty((R, B, k), dtype=np.int32)
    for r in range(R):
        cr = cand[r]  # [B, ncand]
        safe = np.clip(cr, 0, N - 1)
        v = bank[r][safe].astype(np.float64)  # [B, ncand, H]
        nv = np.maximum(np.linalg.norm(v, axis=2), EPS)
        scores = np.einsum("bch,bh->bc", v, qn[:, r]) / nv

        invalid = (cr < 0) | (cr == exclude_idx[:, r, None])
        # order candidates by index so ties resolve to the lowest index,
        # and drop duplicate indices (possible only via exact score ties)
        perm = np.argsort(cr, axis=1, kind="stable")
        cr_s = np.take_along_axis(cr, perm, axis=1)
        sc_s = np.take_along_axis(scores, perm, axis=1)
        inv_s = np.take_along_axis(invalid, perm, axis=1)
        inv_s[:, 1:] |= cr_s[:, 1:] == cr_s[:, :-1]
        sc_s[inv_s] = -np.inf

        order = np.argsort(-sc_s, axis=1, kind="stable")[:, :k]
        top_idx[r] = np.take_along_axis(cr_s, order, axis=1)
        top_sims[r] = np.take_along_axis(sc_s, order, axis=1)

    top_sims = np.ascontiguousarray(top_sims.transpose(1, 0, 2))  # [B, R, k]
    top_idx = np.ascontiguousarray(top_idx.transpose(1, 0, 2))
    best_sims = np.ascontiguousarray(top_sims[:, :, 0])
    return top_sims, top_idx.astype(np.int32), best_sims
